# revision 7
# baseline (speedup 1.0000x reference)
"""Nearest-color-distance loss on 8 TRN2 NeuronCores (kd-pruned bf16 matmul).

loss = mean_i min_j ||x_i - p_j||_2,  x: (131072, 3), p: (128, 3).

Per core (16384 colors): host splits colors into 128 kd-tree leaves of
128, computes per-leaf provably-sufficient palette candidate sets
(dist(box, p_j) <= min_k farthest-corner-dist(box, p_k)), buckets leaf
widths to multiples of 8 using the cross-core max of sorted profiles
(so all 8 cores share one program), and packs leaves block-diagonally
into PSUM banks: stationary [4n, 128] colors (x,1), moving [4n, n*w]
tables (-2p, ||p||^2), one bf16 matmul + one DVE min-reduce (groups of
w) per bank.  Mean candidate count ~26/128 cuts matmul + reduce work
~4x vs brute force.

Feed: banks stack vertically at PE-legal base partitions into dense
contiguous dram rects (lines capped at 608 cols = 1216 B so the DGE
sprays per-row descriptors across all 16 DMA engines), one DMA each,
alternating the two hardware-DGE queues (sync/scalar) in compute
order; results stream back in three pieces.  Host adds ||x||^2, sqrt,
mean in f64.
"""

import sys

sys.path.insert(0, "/opt/trn_rl_repo")

import ml_dtypes
import numpy as np

import concourse.bass as bass
import concourse.tile as tile
from concourse import bacc, mybir
from concourse.alu_op_type import AluOpType
from concourse.bass_utils import run_bass_kernel_spmd

N_CORES = 8
N = 131072
NPC = N // N_CORES  # 16384
M = 128
LEAF = 128
NSLOTS = NPC // LEAF  # 128
BF = ml_dtypes.bfloat16
BF16 = mybir.dt.bfloat16
F32 = mybir.dt.float32

N_MAX = 42        # max slots per bank (K = 3n+1)
MERGE_TH = 384    # fold banks narrower than this many columns
SINGLES = 2       # leading single-bank stacks for a fast start
OUT_SPLIT = 2     # number of result writeback pieces
RANGE_BYTES = 160 * 1024  # target bytes per input DMA range


def _kd_leaves(xs):
    out = []

    def rec(ids):
        if len(ids) == LEAF:
            out.append(ids)
            return
        pts = xs[ids]
        ax = int(np.argmax(pts.max(0) - pts.min(0)))
        h = len(ids) // 2
        part = np.argpartition(pts[:, ax], h)
        rec(ids[part[:h]])
        rec(ids[part[h:]])

    rec(np.arange(len(xs)))
    return out


def _leaf_candidates(xs, leaves, p):
    cands = []
    for ids in leaves:
        pts = xs[ids]
        lo, hi = pts.min(0), pts.max(0)
        clamped = np.clip(p, lo, hi)
        dlo = np.sqrt(((p - clamped) ** 2).sum(1))
        far = np.sqrt(np.maximum((p - lo) ** 2, (p - hi) ** 2).sum(1))
        keep = np.where(dlo <= far.min() + 1e-6)[0]
        cands.append(keep)
    return cands


def _make_layout(counts):
    prof = np.sort(counts, axis=1)[:, ::-1].max(axis=0)
    widths = [min(128, max(8, -8 * (-int(c) // 8))) for c in prof]
    banks = []
    t = 0
    while t < NSLOTS:
        w = widths[t]
        n_max = min(480 // w, N_MAX)
        n = 1
        while t + n < NSLOTS and widths[t + n] == w and n < n_max:
            n += 1
        banks.append([w, n])
        t += n
    changed = True
    while changed and len(banks) > 1:
        changed = False
        for i, (w, n) in enumerate(banks):
            if n * w >= MERGE_TH:
                continue
            for j, (w2, n2) in enumerate(banks):
                if j == i or w2 < w:
                    continue
                if n2 + n <= min(480 // w2, N_MAX) and (w2 - w) * n <= 192:
                    banks[j][1] += n
                    del banks[i]
                    changed = True
                    break
            if changed:
                break
    out_banks = []
    t = 0
    for w, n in banks:
        out_banks.append((w, n, t))
        t += n
    banks = out_banks

    stacks = [[b] for b in range(min(SINGLES, len(banks)))]
    cur = []
    extent = 0
    for b in range(SINGLES, len(banks)):
        K = 3 * banks[b][1] + 1
        base = None
        if cur:
            base = next(
                (x for x in _legal_bases(K) if x >= extent and x + K <= 128),
                None,
            )
        if base is None:
            if cur:
                stacks.append(cur)
            cur, extent = [], 0
            base = 0
        cur.append(b)
        extent = base + K
    if cur:
        stacks.append(cur)
    return widths, banks, stacks


def _legal_bases(K):
    return (0, 32, 64, 96) if K <= 32 else (0, 64) if K <= 64 else (0,)


def _stack_geom(banks, stack, min_width=0):
    extent = 0
    wmax = 0
    places = []
    for b in stack:
        w, n, _ = banks[b]
        K = 3 * n + 1
        r0 = next(x for x in _legal_bases(K) if x >= extent and x + K <= 128)
        places.append((b, r0, 0, 128))
        extent = r0 + K
        wmax = max(wmax, n * w)
    return extent, max(128 + wmax, min_width), places


def build_nc(key):
    banks, stacks, split = key
    nc = bacc.Bacc(
        "TRN2",
        target_bir_lowering=False,
        debug=False,
        enable_asserts=False,
        num_devices=N_CORES,
    )
    geoms = [
        _stack_geom(banks, st, 656 if i == 0 else 0)
        for i, st in enumerate(stacks)
    ]
    dts = [
        nc.dram_tensor(f"st{i}", [rows, width], BF16, kind="ExternalInput").ap()
        for i, (rows, width, _) in enumerate(geoms)
    ]
    cuts = [t0 + n for (w, n, t0) in banks]
    tgt1 = next(c for c in cuts if c >= NSLOTS * 2 // 5)
    tgt2 = next(c for c in cuts if c >= (NSLOTS * 4) // 5)
    mv_cuts = sorted(set([0, tgt1, tgt2, NSLOTS]))
    mv_ds = [
        nc.dram_tensor(
            f"mv{i}", [128, mv_cuts[i + 1] - mv_cuts[i]], F32,
            kind="ExternalOutput",
        ).ap()
        for i in range(len(mv_cuts) - 1)
    ]

    with tile.TileContext(nc) as tc:
        with (
            tc.tile_pool(name="sb", bufs=1) as sb,
            tc.tile_pool(name="pp", bufs=8, space=bass.MemorySpace.PSUM) as pp,
        ):
            # [128, width] tiles with partition-sliced DMA dsts: this is
            # the shape the DGE sprays across all 16 engines (full-tile
            # rects lower to a single descriptor on one engine)
            tiles = [
                sb.tile([128, width], BF16, name=f"stk{i}")
                for i, (rows, width, _) in enumerate(geoms)
            ]
            mv = sb.tile([128, NSLOTS], F32)

            qorder = [nc.sync, nc.scalar]
            bank_ap = {}
            for i, ((rows, width, places), st) in enumerate(zip(geoms, stacks)):
                q = qorder[i % 2]
                q.dma_start(tiles[i][0:rows, :], dts[i][:, :])
                for b, r0, xc, tcc in places:
                    w, n, _ = banks[b]
                    K = 3 * n + 1
                    bank_ap[b] = (
                        tiles[i][r0 : r0 + K, xc : xc + 128],
                        tiles[i][r0 : r0 + K, tcc : tcc + n * w],
                    )

            for b, (w, n, t0) in enumerate(banks):
                stat, mov = bank_ap[b]
                d_ps = pp.tile([128, 512], F32)
                nc.tensor.matmul(
                    d_ps[:, 0 : n * w], stat, mov, start=True, stop=True
                )
                nc.vector.tensor_reduce(
                    mv[:, t0 : t0 + n],
                    d_ps[:, 0 : n * w].rearrange("p (c j) -> p c j", j=w),
                    axis=mybir.AxisListType.X,
                    op=AluOpType.min,
                )
                if t0 + n in mv_cuts[1:-1]:
                    j = mv_cuts.index(t0 + n) - 1
                    nc.sync.dma_start(
                        mv_ds[j][:], mv[:, mv_cuts[j] : mv_cuts[j + 1]]
                    )
            nc.scalar.dma_start(mv_ds[-1][:], mv[:, mv_cuts[-2] :])

    nc.compile()
    return nc


def prep_inputs(output_colors, target_palette):
    pal = np.asarray(target_palette, np.float64)
    mu = pal.mean(0)
    pq = np.asarray(BF(pal - mu), np.float32)
    pn = (pq.astype(np.float64) ** 2).sum(1)
    x = np.asarray(output_colors, np.float64) - mu
    xq = np.asarray(BF(x), np.float32)

    per_core = []
    counts = np.empty((N_CORES, NSLOTS), np.int64)
    for k in range(N_CORES):
        xs = xq[k * NPC : (k + 1) * NPC]
        leaves = _kd_leaves(xs)
        cands = _leaf_candidates(xs, leaves, pq)
        cnt = np.array([len(c) for c in cands])
        order = np.argsort(-cnt, kind="stable")
        counts[k] = cnt[order]
        per_core.append((xs, leaves, cands, order))

    widths, banks, stacks = _make_layout(counts)
    split = next(
        (t0 + n for (w, n, t0) in banks if t0 + n >= NSLOTS // 2), NSLOTS
    )
    geoms = [
        _stack_geom(banks, st, 656 if i == 0 else 0)
        for i, st in enumerate(stacks)
    ]

    in_maps = []
    xn2_slots = np.empty((N_CORES, NSLOTS, LEAF), np.float64)
    for k in range(N_CORES):
        xs, leaves, cands, order = per_core[k]
        amaps = {}
        for i, ((rows, width, places), st) in enumerate(zip(geoms, stacks)):
            arr = np.zeros((rows, width), BF)
            for b, r0, xc, tcc in places:
                w, n, t0 = banks[b]
                arr[r0 + 3 * n, xc : xc + 128] = 1.0
                for s in range(n):
                    leaf = order[t0 + s]
                    ids = leaves[leaf]
                    cd = np.resize(cands[leaf], w)
                    r = r0 + 3 * s
                    arr[r : r + 3, xc : xc + 128] = xs[ids].T
                    cols = slice(tcc + s * w, tcc + (s + 1) * w)
                    arr[r : r + 3, cols] = -2.0 * pq[cd].T
                    arr[r0 + 3 * n, cols] = pn[cd]
                    xn2_slots[k, t0 + s] = (
                        xs[ids].astype(np.float64) ** 2
                    ).sum(1)
            amaps[f"st{i}"] = arr
        in_maps.append(amaps)
    key = (tuple(banks), tuple(tuple(st) for st in stacks), split)
    return key, in_maps, xn2_slots


_NC_CACHE = {}


def get_nc(key):
    if key not in _NC_CACHE:
        _NC_CACHE[key] = build_nc(key)
    return _NC_CACHE[key]


def kernel(output_colors=None, target_palette=None, _trace=False, **_):
    key, in_maps, xn2_slots = prep_inputs(output_colors, target_palette)
    nc = get_nc(key)
    res = run_bass_kernel_spmd(
        nc, in_maps, core_ids=list(range(N_CORES)), trace=_trace
    )
    nmv = sum(1 for kk in res.results[0] if kk.startswith("mv"))
    total = np.float64(0.0)
    for k, r in enumerate(res.results):
        m = np.concatenate([r[f"mv{i}"] for i in range(nmv)], axis=1)
        d2 = m.astype(np.float64).T + xn2_slots[k]
        total += np.sqrt(np.maximum(d2, 0.0)).sum()
    out = np.array(total / N, dtype=np.float32)
    if _trace:
        kernel._last_results = res
    return out


def _selftest():
    rng = np.random.default_rng(0)
    oc = rng.random((N, 3), dtype=np.float32)
    tp = rng.random((M, 3), dtype=np.float32)
    key, in_maps, xn2_slots = prep_inputs(oc, tp)
    banks, stacks, split = key
    print("banks:", banks)
    print("stacks:", stacks, "split:", split)
    print("sum cols:", sum(n * w for w, n, _ in banks))
    geoms = [
        _stack_geom(banks, st, 656 if i == 0 else 0)
        for i, st in enumerate(stacks)
    ]
    print("dma bytes:", sum(r * w * 2 for (r, w, _) in geoms))

    total = 0.0
    for k in range(N_CORES):
        m = np.empty((128, NSLOTS), np.float32)
        for i, ((rows, width, places), st) in enumerate(zip(geoms, stacks)):
            arr = np.asarray(in_maps[k][f"st{i}"], np.float32)
            for b, r0, xc, tcc in places:
                w, n, t0 = banks[b]
                K = 3 * n + 1
                stat = arr[r0 : r0 + K, xc : xc + 128]
                mov = arr[r0 : r0 + K, tcc : tcc + n * w]
                out = stat.T @ mov
                m[:, t0 : t0 + n] = out.reshape(128, n, w).min(axis=2)
        d2 = m.astype(np.float64).T + xn2_slots[k]
        total += np.sqrt(np.maximum(d2, 0.0)).sum()
    got = total / N
    d = oc[:, None, :].astype(np.float64) - tp[None, :, :].astype(np.float64)
    want = np.sqrt((d * d).sum(-1)).min(1).mean()
    print("got", got, "want", want, "rel", abs(got - want) / want)


if __name__ == "__main__":
    _selftest()
